# revision 1
# baseline (speedup 1.0000x reference)
"""BayesianLinear Trainium2 kernel, 8-core SPMD (data-parallel over batch).

Per-core computation (4 samples each):
    w_b = weight_mean + noise_b * exp(0.5 * weight_logvar)   (B,O,I)
    out_b = x_b @ w_b^T + bias                               (B,L,O)

Design (per core):
  - std = exp(0.5*logvar) once on ACT, kept natural (O on partitions).
  - mean^T resident (PE-transposed once at startup).
  - Per sample, software-pipelined in column halves so PE stays dense:
      [x load + PE-transpose x (ACT rounds to fp32r on evac)]
      [noise chunks 0,1: DVE/GpSimd scale-mul (fp32r round), PE transpose,
       DVE fused mean-add evac -> w^T cols 0-511]
      [matmuls n=0: psum += x^T.T @ w^T over 8 k-tiles, K=1 bias matmul,
       ACT evac, store]
      [chunks 2,3 -> w^T cols 512-1023]  [matmuls n=1]
  - fp32r matmuls run the PE at bf16 rate (1 cyc/row); fp32r transposes are
    exact permutations; rounding (~2^-12) happens once per operand.
"""
import numpy as np

SAMPLES = 4           # batch samples per core
N_CORES = 8
B, L, I, O = 32, 512, 1024, 1024
KT = I // 128         # 8 k-tiles (contraction)
OT = O // 128         # 8 o-blocks
LT = L // 128         # 4 l-tiles
NCH = 4               # noise chunks per sample (2 o-blocks each)

_cache = {}


def _split_multi_waits(nc, mybir):
    """This walrus build allows at most one sync-wait per instruction; move
    extra waits onto preceding single-wait NOPs on the same engine.  Safe
    because kernel semaphores are monotonic between resets, so waiting
    sequentially is equivalent to waiting on the conjunction."""
    for fn in nc.m.functions:
        for bb in fn.blocks:
            insts = bb.instructions
            changed = False
            new_list = []
            for inst in insts:
                si = inst.sync_info
                if si is not None and si.on_wait and len(si.on_wait) > 1:
                    waits = list(si.on_wait)
                    for j, w in enumerate(waits[:-1]):
                        nop = mybir.InstNoOp(name=f"{inst.name}-w{j}", ins=[], outs=[])
                        nop.engine = inst.engine
                        nop.sync_info = mybir.SyncInfo(on_wait=[w], on_update=[])
                        new_list.append(nop)
                    inst.sync_info = mybir.SyncInfo(
                        on_wait=[waits[-1]], on_update=list(si.on_update or []))
                    changed = True
                new_list.append(inst)
            if changed:
                bb.instructions = new_list


def build_nc(use_f32r=True):
    from contextlib import ExitStack
    from concourse import bass, mybir, tile, masks

    F32 = mybir.dt.float32
    F32R = mybir.dt.float32r if use_f32r else mybir.dt.float32
    Exp = mybir.ActivationFunctionType.Exp
    Copy = mybir.ActivationFunctionType.Copy
    mult = mybir.AluOpType.mult
    add = mybir.AluOpType.add

    nc = bass.Bass()
    x_d = nc.declare_dram_parameter("x", [SAMPLES, L, I], F32, isOutput=False)
    nz_d = nc.declare_dram_parameter("noise", [SAMPLES, O, I], F32, isOutput=False)
    wm_d = nc.declare_dram_parameter("weight_mean", [O, I], F32, isOutput=False)
    wl_d = nc.declare_dram_parameter("weight_logvar", [O, I], F32, isOutput=False)
    b_d = nc.declare_dram_parameter("bias", [O], F32, isOutput=False)
    out_d = nc.declare_dram_parameter("out", [SAMPLES, L, O], F32, isOutput=True)

    with tile.TileContext(nc) as tc, ExitStack() as ctx:
        resident = ctx.enter_context(tc.tile_pool(name="resident", bufs=1))
        nat_pool = ctx.enter_context(tc.tile_pool(name="nat", bufs=3))
        scn_pool = ctx.enter_context(tc.tile_pool(name="scn", bufs=2))
        xnat_pool = ctx.enter_context(tc.tile_pool(name="xnat", bufs=2))
        wx_pool = ctx.enter_context(tc.tile_pool(name="wx", bufs=1))
        out_pool = ctx.enter_context(tc.tile_pool(name="outp", bufs=3))
        psum_mm = ctx.enter_context(tc.tile_pool(name="psum_mm", bufs=2, space="PSUM"))
        psum_nt = ctx.enter_context(tc.tile_pool(name="psum_nt", bufs=3, space="PSUM"))
        psum_xt = ctx.enter_context(tc.tile_pool(name="psum_xt", bufs=3, space="PSUM"))

        # ---------------- one-time setup ----------------
        std_nat = resident.tile([128, OT, I], F32, tag="std")    # exp(.5 lv), natural
        meanT = resident.tile([128, KT, O], F32, tag="meanT")    # mean^T
        ident = resident.tile([128, 128], F32, tag="ident")
        ident_r = resident.tile([128, 128], F32R, tag="ident_r")
        ones_f = resident.tile([1, 128], F32, tag="ones_f")
        ones_r = resident.tile([1, 128], F32R, tag="ones_r")
        bias_f = resident.tile([1, O], F32, tag="bias_f")
        bias_r = resident.tile([1, O], F32R, tag="bias_r")

        masks.make_identity(nc, ident[:])
        nc.vector.tensor_copy(ident_r[:], ident[:])
        nc.vector.memset(ones_f[:], 1.0)
        nc.vector.tensor_copy(ones_r[:], ones_f[:])
        nc.sync.dma_start(bias_f[:], b_d[:].rearrange("(a o) -> a o", a=1))
        nc.vector.tensor_copy(bias_r[:], bias_f[:])

        # PE pre-warm: short burst of dummy transposes (self as moving operand,
        # result is garbage, never read) trips the HAM activity window so the
        # 2.4 GHz clock is ungated when real work arrives.
        warm = resident.tile([128, 128], F32, tag="warm")
        nc.gpsimd.memset(warm[:], 0.001)
        pwarm = psum_xt.tile([128, 4, 128], F32, tag="pxt")
        for _ in range(16):
            nc.tensor.matmul(pwarm[:, 0, :], warm[:], warm[:],
                             is_transpose=True, start=True, stop=True)

        def emit_mean_slab(j):
            """load + exp + transpose weight slab j (o-blocks 2j, 2j+1)."""
            sl = slice(256 * j, 256 * (j + 1))
            mt = nat_pool.tile([128, 2, I], F32, tag="nat", name=f"mt{j}")
            nc.sync.dma_start(
                mt[:], wm_d[sl, :].rearrange("(q p) i -> p q i", p=128))
            nc.sync.dma_start(std_nat[:, 2 * j:2 * (j + 1), :],
                              wl_d[sl, :].rearrange("(q p) i -> p q i", p=128))
            nc.scalar.activation(std_nat[:, 2 * j:2 * (j + 1), :],
                                 std_nat[:, 2 * j:2 * (j + 1), :],
                                 Exp, bias=0.0, scale=0.5)
            for q in range(2):
                ob = 2 * j + q
                for kh in range(2):  # k halves of 4
                    px = psum_xt.tile([128, 4, 128], F32, tag="pxt")
                    for kk in range(4):
                        k = 4 * kh + kk
                        nc.tensor.matmul(
                            px[:, kk, :], mt[:, q, 128 * k:128 * (k + 1)], ident[:],
                            is_transpose=True, start=True, stop=True)
                    nc.vector.tensor_copy(
                        meanT[:, 4 * kh:4 * (kh + 1), 128 * ob:128 * (ob + 1)], px[:])

        # ---------------- per-sample pipeline ----------------
        def emit_chunk(b, c, wT):
            """noise chunk c (o-blocks 2c, 2c+1): load, scale, transpose, add."""
            nz = nat_pool.tile([128, 2, I], F32, tag="nat")
            nc.sync.dma_start(
                nz[:], nz_d[b, 256 * c:256 * (c + 1), :].rearrange("(q p) i -> p q i", p=128))
            sc = scn_pool.tile([128, 2, I], F32R, tag="scn")
            # alternate scale-muls between DVE and GpSimd
            eng = nc.gpsimd if c == 3 else nc.vector
            eng.tensor_tensor(sc[:], nz[:], std_nat[:, 2 * c:2 * (c + 1), :], mult)
            for q in range(2):
                ob = 2 * c + q
                for kh in range(2):
                    pn = psum_nt.tile([128, 4, 128], F32R, tag="pnt")
                    for kk in range(4):
                        k = 4 * kh + kk
                        nc.tensor.matmul(
                            pn[:, kk, :], sc[:, q, 128 * k:128 * (k + 1)],
                            ident_r[:], is_transpose=True, start=True, stop=True)
                    nc.vector.tensor_tensor(
                        wT[:, 4 * kh:4 * (kh + 1), 128 * ob:128 * (ob + 1)],
                        pn[:], meanT[:, 4 * kh:4 * (kh + 1), 128 * ob:128 * (ob + 1)],
                        add)

        def emit_mm_half(b, n, wT, xT):
            """matmuls for output columns [512n, 512(n+1))."""
            for m in range(LT):
                pm = psum_mm.tile([128, 512], F32, tag="pmm")
                for k in range(KT):
                    nc.tensor.matmul(pm[:], xT[:, k, 128 * m:128 * (m + 1)],
                                     wT[:, k, 512 * n:512 * (n + 1)],
                                     start=(k == 0), stop=False)
                nc.tensor.matmul(pm[:], ones_r[:], bias_r[:, 512 * n:512 * (n + 1)],
                                 start=False, stop=True)
                ot = out_pool.tile([128, 512], F32, tag="out")
                nc.scalar.activation(ot[:], pm[:], Copy)
                nc.scalar.dma_start(
                    out_d[b, 128 * m:128 * (m + 1), 512 * n:512 * (n + 1)], ot[:])

        x_tiles = {0: xnat_pool.tile([128, LT, I], F32, tag="xnat", name="xn0")}
        nc.sync.dma_start(x_tiles[0][:], x_d[0].rearrange("(m p) i -> p m i", p=128))
        for b in range(SAMPLES):
            # x^T build (ACT rounds on evac)
            xT = wx_pool.tile([128, KT, L], F32R, tag="xT")
            x_nat = x_tiles.pop(b)
            for m in range(LT):
                for kh in range(2):
                    px = psum_xt.tile([128, 4, 128], F32, tag="pxt")
                    for kk in range(4):
                        k = 4 * kh + kk
                        nc.tensor.matmul(
                            px[:, kk, :], x_nat[:, m, 128 * k:128 * (k + 1)], ident[:],
                            is_transpose=True, start=True, stop=True)
                    nc.scalar.activation(
                        xT[:, 4 * kh:4 * (kh + 1), 128 * m:128 * (m + 1)], px[:], Copy)
            if b + 1 < SAMPLES:
                xn_next = xnat_pool.tile([128, LT, I], F32, tag="xnat", name=f"xn{b+1}")
                x_tiles[b + 1] = xn_next
                nc.sync.dma_start(
                    x_tiles[b + 1][:], x_d[b + 1].rearrange("(m p) i -> p m i", p=128))
            wT = wx_pool.tile([128, KT, O], F32R, tag="wT")
            for half in range(2):
                for cc in (2 * half, 2 * half + 1):
                    if b == 0:
                        emit_mean_slab(cc)
                    emit_chunk(b, cc, wT)
                emit_mm_half(b, half, wT, xT)

    _split_multi_waits(nc, mybir)
    return nc


def _get_nc(use_f32r=True):
    key = ("nc", use_f32r)
    if key not in _cache:
        _cache[key] = build_nc(use_f32r)
    return _cache[key]


def kernel(x, weight_mean, weight_logvar, bias, noise):
    from concourse import bass_utils

    x = np.ascontiguousarray(x, dtype=np.float32)
    noise = np.ascontiguousarray(noise, dtype=np.float32)
    weight_mean = np.ascontiguousarray(weight_mean, dtype=np.float32)
    weight_logvar = np.ascontiguousarray(weight_logvar, dtype=np.float32)
    bias = np.ascontiguousarray(bias, dtype=np.float32)

    nc = _get_nc()
    in_maps = []
    for c in range(N_CORES):
        sl = slice(SAMPLES * c, SAMPLES * (c + 1))
        in_maps.append({
            "x": x[sl], "noise": noise[sl],
            "weight_mean": weight_mean, "weight_logvar": weight_logvar,
            "bias": bias,
        })
    res = bass_utils.run_bass_kernel_spmd(nc, in_maps, list(range(N_CORES)))
    out = np.concatenate([res.results[c]["out"] for c in range(N_CORES)], axis=0)
    return out.astype(np.float32)



# revision 2
# speedup vs baseline: 1.0574x; 1.0574x over previous
"""BayesianLinear Trainium2 kernel, 8-core SPMD (data-parallel over batch).

Per-core computation (4 samples each):
    w_b = weight_mean + noise_b * exp(0.5 * weight_logvar)   (B,O,I)
    out_b = x_b @ w_b^T + bias                               (B,L,O)

Design (per core) — DMA-roofline oriented (~40 MB/core HBM traffic):
  - All matmul operands are bf16 (tolerance 2e-2; bf16 path lands ~4e-3):
    noise/x/mean are cast f32->bf16 *during* the DMA load (SWDGE cast),
    std = exp(0.5*logvar) is produced in bf16 by ACT directly.
  - bf16 makes PE transposes ~2x cheaper (FWL weight loads) and DVE
    elementwise 2x faster (2x_1P mode); GEMM accumulates in f32 PSUM.
  - bias is pre-broadcast once into a resident [128, O] block and
    preloaded into each GEMM PSUM tile by a cheap ident matmul
    (replaces the slow per-tile K=1 bias matmuls).
  - Per sample, software-pipelined in column halves so PE stays dense:
      [x^T build]  [noise chunks 0,1 -> w^T cols 0-511]  [matmuls half 0]
      [chunks 2,3 -> w^T cols 512-1023]  [matmuls half 1]
"""
import numpy as np

SAMPLES = 4           # batch samples per core
N_CORES = 8
B, L, I, O = 32, 512, 1024, 1024
KT = I // 128         # 8 k-tiles (contraction)
OT = O // 128         # 8 o-blocks
LT = L // 128         # 4 l-tiles
NCH = 4               # noise chunks per sample (2 o-blocks each)

_cache = {}


def _split_multi_waits(nc, mybir):
    """This walrus build allows at most one sync-wait per instruction; move
    extra waits onto preceding single-wait NOPs on the same engine.  Safe
    because kernel semaphores are monotonic between resets, so waiting
    sequentially is equivalent to waiting on the conjunction."""
    for fn in nc.m.functions:
        for bb in fn.blocks:
            insts = bb.instructions
            changed = False
            new_list = []
            for inst in insts:
                si = inst.sync_info
                if si is not None and si.on_wait and len(si.on_wait) > 1:
                    waits = list(si.on_wait)
                    for j, w in enumerate(waits[:-1]):
                        nop = mybir.InstNoOp(name=f"{inst.name}-w{j}", ins=[], outs=[])
                        nop.engine = inst.engine
                        nop.sync_info = mybir.SyncInfo(on_wait=[w], on_update=[])
                        new_list.append(nop)
                    inst.sync_info = mybir.SyncInfo(
                        on_wait=[waits[-1]], on_update=list(si.on_update or []))
                    changed = True
                new_list.append(inst)
            if changed:
                bb.instructions = new_list


def build_nc(use_f32r=True):
    from contextlib import ExitStack
    from concourse import bass, mybir, tile, masks

    F32 = mybir.dt.float32
    BF16 = mybir.dt.bfloat16
    Exp = mybir.ActivationFunctionType.Exp
    Copy = mybir.ActivationFunctionType.Copy
    mult = mybir.AluOpType.mult
    add = mybir.AluOpType.add

    nc = bass.Bass()
    x_d = nc.declare_dram_parameter("x", [SAMPLES, L, I], F32, isOutput=False)
    nz_d = nc.declare_dram_parameter("noise", [SAMPLES, O, I], F32, isOutput=False)
    wm_d = nc.declare_dram_parameter("weight_mean", [O, I], F32, isOutput=False)
    wl_d = nc.declare_dram_parameter("weight_logvar", [O, I], F32, isOutput=False)
    b_d = nc.declare_dram_parameter("bias", [O], F32, isOutput=False)
    out_d = nc.declare_dram_parameter("out", [SAMPLES, L, O], F32, isOutput=True)

    with tile.TileContext(nc) as tc, ExitStack() as ctx:
        resident = ctx.enter_context(tc.tile_pool(name="resident", bufs=1))
        lv_pool = ctx.enter_context(tc.tile_pool(name="lv", bufs=2))
        mn_pool = ctx.enter_context(tc.tile_pool(name="mn", bufs=2))
        nz_pool = ctx.enter_context(tc.tile_pool(name="nz", bufs=3))
        sc_pool = ctx.enter_context(tc.tile_pool(name="scn", bufs=2))
        xnat_pool = ctx.enter_context(tc.tile_pool(name="xnat", bufs=2))
        xT_pool = ctx.enter_context(tc.tile_pool(name="xT", bufs=2))
        wT_pool = ctx.enter_context(tc.tile_pool(name="wT", bufs=2))
        out_pool = ctx.enter_context(tc.tile_pool(name="outp", bufs=3))
        psum_mm = ctx.enter_context(tc.tile_pool(name="psum_mm", bufs=2, space="PSUM"))
        psum_nt = ctx.enter_context(tc.tile_pool(name="psum_nt", bufs=3, space="PSUM"))
        psum_xt = ctx.enter_context(tc.tile_pool(name="psum_xt", bufs=2, space="PSUM"))
        psum_wm = ctx.enter_context(tc.tile_pool(name="psum_wm", bufs=1, space="PSUM"))

        # ---------------- one-time setup ----------------
        std_b = resident.tile([128, OT, I], BF16, tag="std")     # exp(.5 lv), natural
        meanT = resident.tile([128, KT, O], BF16, tag="meanT")   # mean^T
        ident = resident.tile([128, 128], F32, tag="ident")
        ident_b = resident.tile([128, 128], BF16, tag="ident_b")
        ones_b = resident.tile([1, 128], BF16, tag="ones_b")
        bias_f = resident.tile([1, O], F32, tag="bias_f")
        bias_b = resident.tile([1, O], BF16, tag="bias_b")
        bias_blk = resident.tile([128, O], BF16, tag="bias_blk")  # bias bcast to rows
        warm_sb = resident.tile([128, 128], BF16, tag="warm")

        masks.make_identity(nc, ident[:])
        nc.vector.tensor_copy(ident_b[:], ident[:])
        nc.vector.memset(ones_b[:], 1.0)
        nc.gpsimd.memset(warm_sb[:], 0.001)
        nc.sync.dma_start(bias_f[:], b_d[:].rearrange("(a o) -> a o", a=1))
        nc.vector.tensor_copy(bias_b[:], bias_f[:])

        # bias broadcast block: ones^T (x) bias, evac'd bf16
        for n in range(2):
            pb = psum_mm.tile([128, 512], F32, tag="pmm")
            nc.tensor.matmul(pb[:], ones_b[:], bias_b[:, 512 * n:512 * (n + 1)],
                             start=True, stop=True)
            nc.vector.tensor_copy(bias_blk[:, 512 * n:512 * (n + 1)], pb[:])

        # PE pre-warm: dummy non-transpose matmuls trip the HAM activity
        # window so the 2.4 GHz clock is ungated when real work arrives.
        pwarm = psum_wm.tile([128, 128], F32, tag="pwm")
        for _ in range(34):
            nc.tensor.matmul(pwarm[:], warm_sb[:], warm_sb[:], start=True, stop=True)

        def emit_mean_slab(j):
            """load + transpose mean slab j; exp logvar slab j (o-blocks 2j,2j+1)."""
            sl = slice(256 * j, 256 * (j + 1))
            lv = lv_pool.tile([128, 2, I], F32, tag="lv")
            nc.sync.dma_start(lv[:], wl_d[sl, :].rearrange("(q p) i -> p q i", p=128))
            nc.scalar.activation(std_b[:, 2 * j:2 * (j + 1), :], lv[:],
                                 Exp, bias=0.0, scale=0.5)
            mn = mn_pool.tile([128, 2, I], BF16, tag="mn")
            nc.gpsimd.dma_start(mn[:], wm_d[sl, :].rearrange("(q p) i -> p q i", p=128))
            for q in range(2):
                ob = 2 * j + q
                for kh in range(2):
                    px = psum_nt.tile([128, 4, 128], BF16, tag="pnt")
                    for kk in range(4):
                        k = 4 * kh + kk
                        nc.tensor.matmul(
                            px[:, kk, :], mn[:, q, 128 * k:128 * (k + 1)], ident_b[:],
                            is_transpose=True, start=True, stop=True)
                    nc.vector.tensor_copy(
                        meanT[:, 4 * kh:4 * (kh + 1), 128 * ob:128 * (ob + 1)], px[:])

        # ---------------- per-sample pipeline ----------------
        def emit_chunk(b, c, wT):
            """noise chunk c (o-blocks 2c, 2c+1): cast-load, scale, transpose,
            add mean^T."""
            nz = nz_pool.tile([128, 2, I], BF16, tag="nz")
            nc.gpsimd.dma_start(
                nz[:], nz_d[b, 256 * c:256 * (c + 1), :].rearrange("(q p) i -> p q i", p=128))
            sc = sc_pool.tile([128, 2, I], BF16, tag="scn")
            nc.vector.tensor_tensor(sc[:], nz[:], std_b[:, 2 * c:2 * (c + 1), :], mult)
            for q in range(2):
                ob = 2 * c + q
                for kh in range(2):
                    pn = psum_nt.tile([128, 4, 128], BF16, tag="pnt")
                    for kk in range(4):
                        k = 4 * kh + kk
                        nc.tensor.matmul(
                            pn[:, kk, :], sc[:, q, 128 * k:128 * (k + 1)],
                            ident_b[:], is_transpose=True, start=True, stop=True)
                    nc.vector.tensor_tensor(
                        wT[:, 4 * kh:4 * (kh + 1), 128 * ob:128 * (ob + 1)],
                        pn[:], meanT[:, 4 * kh:4 * (kh + 1), 128 * ob:128 * (ob + 1)],
                        add)

        def emit_mm_half(b, n, wT, xT):
            """matmuls for output columns [512n, 512(n+1))."""
            ot = out_pool.tile([128, LT, 512], F32, tag="out")
            for m in range(LT):
                pm = psum_mm.tile([128, 512], F32, tag="pmm")
                nc.tensor.matmul(pm[:], ident_b[:], bias_blk[:, 512 * n:512 * (n + 1)],
                                 start=True, stop=False)
                for k in range(KT):
                    nc.tensor.matmul(pm[:], xT[:, k, 128 * m:128 * (m + 1)],
                                     wT[:, k, 512 * n:512 * (n + 1)],
                                     start=False, stop=(k == KT - 1))
                nc.scalar.activation(ot[:, m, :], pm[:], Copy)
            nc.scalar.dma_start(
                out_d[b, :, 512 * n:512 * (n + 1)].rearrange("(m p) o -> p m o", p=128),
                ot[:])

        x_tiles = {0: xnat_pool.tile([128, LT, I], BF16, tag="xnat", name="xn0")}
        nc.gpsimd.dma_start(x_tiles[0][:], x_d[0].rearrange("(m p) i -> p m i", p=128))
        for b in range(SAMPLES):
            # x^T build (bf16 transposes; ACT evacs)
            xT = xT_pool.tile([128, KT, L], BF16, tag="xT")
            x_nat = x_tiles.pop(b)
            for m in range(LT):
                for kh in range(2):
                    px = psum_xt.tile([128, 4, 128], BF16, tag="pxt")
                    for kk in range(4):
                        k = 4 * kh + kk
                        nc.tensor.matmul(
                            px[:, kk, :], x_nat[:, m, 128 * k:128 * (k + 1)], ident_b[:],
                            is_transpose=True, start=True, stop=True)
                    nc.scalar.activation(
                        xT[:, 4 * kh:4 * (kh + 1), 128 * m:128 * (m + 1)], px[:], Copy)
            if b + 1 < SAMPLES:
                xn_next = xnat_pool.tile([128, LT, I], BF16, tag="xnat", name=f"xn{b+1}")
                x_tiles[b + 1] = xn_next
                nc.gpsimd.dma_start(
                    x_tiles[b + 1][:], x_d[b + 1].rearrange("(m p) i -> p m i", p=128))
            wT = wT_pool.tile([128, KT, O], BF16, tag="wT")
            for half in range(2):
                for cc in (2 * half, 2 * half + 1):
                    if b == 0:
                        emit_mean_slab(cc)
                    emit_chunk(b, cc, wT)
                emit_mm_half(b, half, wT, xT)

    _split_multi_waits(nc, mybir)
    return nc


def _get_nc(use_f32r=True):
    key = ("nc", use_f32r)
    if key not in _cache:
        _cache[key] = build_nc(use_f32r)
    return _cache[key]


def kernel(x, weight_mean, weight_logvar, bias, noise):
    from concourse import bass_utils

    x = np.ascontiguousarray(x, dtype=np.float32)
    noise = np.ascontiguousarray(noise, dtype=np.float32)
    weight_mean = np.ascontiguousarray(weight_mean, dtype=np.float32)
    weight_logvar = np.ascontiguousarray(weight_logvar, dtype=np.float32)
    bias = np.ascontiguousarray(bias, dtype=np.float32)

    nc = _get_nc()
    in_maps = []
    for c in range(N_CORES):
        sl = slice(SAMPLES * c, SAMPLES * (c + 1))
        in_maps.append({
            "x": x[sl], "noise": noise[sl],
            "weight_mean": weight_mean, "weight_logvar": weight_logvar,
            "bias": bias,
        })
    res = bass_utils.run_bass_kernel_spmd(nc, in_maps, list(range(N_CORES)))
    out = np.concatenate([res.results[c]["out"] for c in range(N_CORES)], axis=0)
    return out.astype(np.float32)


# revision 3
# speedup vs baseline: 1.2026x; 1.1373x over previous
"""BayesianLinear Trainium2 kernel, 8-core SPMD (data-parallel over batch).

Per-core computation (4 samples each):
    w_b = weight_mean + noise_b * exp(0.5 * weight_logvar)   (B,O,I)
    out_b = x_b @ w_b^T + bias                               (B,L,O)

Design (per core) — DMA-roofline oriented (~40 MB/core HBM traffic):
  - All matmul operands are bf16 (tolerance 2e-2; bf16 path lands ~4e-3):
    noise/x/mean are cast f32->bf16 *during* the DMA load (SWDGE cast),
    std = exp(0.5*logvar) is produced in bf16 by ACT directly.
  - bf16 PE transposes run at ~67 ns/block (FWL); grouped 8-deep per PSUM
    bank so each evac/add is one [128,8,128] op.
  - GEMM: per l-tile one 2-bank PSUM tile; bias is preloaded by ident
    matmuls, then each xT stationary is shared by two N=512 matmuls
    (both output halves) halving LDWEIGHTS traffic.
  - Software pipelining: sample b+1's transpose groups are interleaved
    between sample b's GEMM l-tiles so the PE never has a >2us stretch
    without real matmuls (keeps the HAM clock at 2.4 GHz).
"""
import numpy as np

SAMPLES = 4           # batch samples per core
N_CORES = 8
B, L, I, O = 32, 512, 1024, 1024
KT = I // 128         # 8 k-tiles (contraction)
OT = O // 128         # 8 o-blocks
LT = L // 128         # 4 l-tiles
NCH = 4               # noise chunks per sample (2 o-blocks each)

_cache = {}


def _split_multi_waits(nc, mybir):
    """This walrus build allows at most one sync-wait per instruction; move
    extra waits onto preceding single-wait NOPs on the same engine.  Safe
    because kernel semaphores are monotonic between resets, so waiting
    sequentially is equivalent to waiting on the conjunction."""
    for fn in nc.m.functions:
        for bb in fn.blocks:
            insts = bb.instructions
            changed = False
            new_list = []
            for inst in insts:
                si = inst.sync_info
                if si is not None and si.on_wait and len(si.on_wait) > 1:
                    waits = list(si.on_wait)
                    for j, w in enumerate(waits[:-1]):
                        nop = mybir.InstNoOp(name=f"{inst.name}-w{j}", ins=[], outs=[])
                        nop.engine = inst.engine
                        nop.sync_info = mybir.SyncInfo(on_wait=[w], on_update=[])
                        new_list.append(nop)
                    inst.sync_info = mybir.SyncInfo(
                        on_wait=[waits[-1]], on_update=list(si.on_update or []))
                    changed = True
                new_list.append(inst)
            if changed:
                bb.instructions = new_list


def build_nc(use_f32r=True):
    from contextlib import ExitStack
    from concourse import bass, mybir, tile, masks

    F32 = mybir.dt.float32
    BF16 = mybir.dt.bfloat16
    Exp = mybir.ActivationFunctionType.Exp
    Copy = mybir.ActivationFunctionType.Copy
    mult = mybir.AluOpType.mult
    add = mybir.AluOpType.add

    nc = bass.Bass()
    x_d = nc.declare_dram_parameter("x", [SAMPLES, L, I], F32, isOutput=False)
    nz_d = nc.declare_dram_parameter("noise", [SAMPLES, O, I], F32, isOutput=False)
    wm_d = nc.declare_dram_parameter("weight_mean", [O, I], F32, isOutput=False)
    wl_d = nc.declare_dram_parameter("weight_logvar", [O, I], F32, isOutput=False)
    b_d = nc.declare_dram_parameter("bias", [O], F32, isOutput=False)
    out_d = nc.declare_dram_parameter("out", [SAMPLES, L, O], F32, isOutput=True)

    with tile.TileContext(nc) as tc, ExitStack() as ctx:
        resident = ctx.enter_context(tc.tile_pool(name="resident", bufs=1))
        lv_pool = ctx.enter_context(tc.tile_pool(name="lv", bufs=2))
        mn_pool = ctx.enter_context(tc.tile_pool(name="mn", bufs=2))
        nz_pool = ctx.enter_context(tc.tile_pool(name="nz", bufs=4))
        sc_pool = ctx.enter_context(tc.tile_pool(name="scn", bufs=2))
        xnat_pool = ctx.enter_context(tc.tile_pool(name="xnat", bufs=2))
        xT_pool = ctx.enter_context(tc.tile_pool(name="xT", bufs=2))
        wT_pool = ctx.enter_context(tc.tile_pool(name="wT", bufs=2))
        out_pool = ctx.enter_context(tc.tile_pool(name="outp", bufs=2))
        psum_mm = ctx.enter_context(tc.tile_pool(name="psum_mm", bufs=2, space="PSUM"))
        psum_nt = ctx.enter_context(tc.tile_pool(name="psum_nt", bufs=2, space="PSUM"))
        psum_xt = ctx.enter_context(tc.tile_pool(name="psum_xt", bufs=2, space="PSUM"))

        # ---------------- residents ----------------
        std_b = resident.tile([128, OT, I], BF16, tag="std")     # exp(.5 lv), natural
        meanT = resident.tile([128, KT, O], BF16, tag="meanT")   # mean^T
        ident = resident.tile([128, 128], F32, tag="ident")
        ident_b = resident.tile([128, 128], BF16, tag="ident_b")
        ones_b = resident.tile([1, 128], BF16, tag="ones_b")
        bias_f = resident.tile([1, O], F32, tag="bias_f")
        bias_b = resident.tile([1, O], BF16, tag="bias_b")
        bias_blk = resident.tile([128, O], BF16, tag="bias_blk")  # bias bcast to rows

        # ---------------- earliest DMA issues ----------------
        x_tiles = {0: xnat_pool.tile([128, LT, I], BF16, tag="xnat", name="xn0")}
        nc.gpsimd.dma_start(x_tiles[0][:], x_d[0].rearrange("(m p) i -> p m i", p=128))
        nc.sync.dma_start(bias_f[:], b_d[:].rearrange("(a o) -> a o", a=1))

        mn_tiles, nz_tiles, lv_tiles = {}, {}, {}
        for j in range(NCH):  # sample-0 weights + noise, interleaved by need
            lv_tiles[j] = lv_pool.tile([128, 2, I], F32, tag="lv", name=f"lv{j}")
            nc.sync.dma_start(
                lv_tiles[j][:],
                wl_d[256 * j:256 * (j + 1), :].rearrange("(q p) i -> p q i", p=128))
            mn_tiles[j] = mn_pool.tile([128, 2, I], BF16, tag="mn", name=f"mn{j}")
            nc.gpsimd.dma_start(
                mn_tiles[j][:],
                wm_d[256 * j:256 * (j + 1), :].rearrange("(q p) i -> p q i", p=128))
            nz_tiles[(0, j)] = nz_pool.tile([128, 2, I], BF16, tag="nz", name=f"nz0{j}")
            nc.gpsimd.dma_start(
                nz_tiles[(0, j)][:],
                nz_d[0, 256 * j:256 * (j + 1), :].rearrange("(q p) i -> p q i", p=128))

        # ---------------- setup compute ----------------
        masks.make_identity(nc, ident[:])
        nc.vector.tensor_copy(ident_b[:], ident[:])
        nc.vector.memset(ones_b[:], 1.0)
        nc.vector.tensor_copy(bias_b[:], bias_f[:])

        # ---------------- emitters ----------------
        def emit_mean_group(j, q):
            """transpose mean slab j, column-half q -> meanT o-block 2j+q."""
            ob = 2 * j + q
            mn = mn_tiles[j]
            pt = psum_nt.tile([128, KT, 128], BF16, tag="pnt")
            for k in range(KT):
                nc.tensor.matmul(pt[:, k, :], mn[:, q, 128 * k:128 * (k + 1)],
                                 ident_b[:], is_transpose=True, start=True, stop=True)
            nc.vector.tensor_copy(meanT[:, :, 128 * ob:128 * (ob + 1)], pt[:])

        def emit_exp(j):
            nc.scalar.activation(std_b[:, 2 * j:2 * (j + 1), :], lv_tiles.pop(j)[:],
                                 Exp, bias=0.0, scale=0.5)

        sc_tiles = {}

        def emit_scale(b, c):
            """sc = noise_chunk * std (bf16)."""
            nz = nz_tiles.pop((b, c))
            sc = sc_pool.tile([128, 2, I], BF16, tag="scn")
            nc.vector.tensor_tensor(sc[:], nz[:], std_b[:, 2 * c:2 * (c + 1), :], mult)
            sc_tiles[(b, c)] = sc

        def emit_chunk_group(b, c, q, wT):
            """transpose sc chunk c half q, add mean^T -> wT o-block 2c+q."""
            ob = 2 * c + q
            sc = sc_tiles[(b, c)] if q == 0 else sc_tiles.pop((b, c))
            pt = psum_nt.tile([128, KT, 128], BF16, tag="pnt")
            for k in range(KT):
                nc.tensor.matmul(pt[:, k, :], sc[:, q, 128 * k:128 * (k + 1)],
                                 ident_b[:], is_transpose=True, start=True, stop=True)
            nc.vector.tensor_tensor(wT[:, :, 128 * ob:128 * (ob + 1)], pt[:],
                                    meanT[:, :, 128 * ob:128 * (ob + 1)], add)

        def emit_xT_group(b, m, x_nat, xT):
            """transpose x l-tile m -> xT[:, :, 128m:128(m+1)]."""
            pt = psum_xt.tile([128, KT, 128], BF16, tag="pxt")
            for k in range(KT):
                nc.tensor.matmul(pt[:, k, :], x_nat[:, m, 128 * k:128 * (k + 1)],
                                 ident_b[:], is_transpose=True, start=True, stop=True)
            nc.scalar.activation(xT[:, :, 128 * m:128 * (m + 1)], pt[:], Copy)

        def emit_gemm_tile(b, m, wT, xT, ot):
            """output l-tile m, all 1024 columns: bias preload + 8 shared-
            stationary k-steps, two N=512 matmuls (psum banks) per step."""
            pm = psum_mm.tile([128, 2, 512], F32, tag="pmm")
            for n in range(2):
                nc.tensor.matmul(pm[:, n, :], ident_b[:],
                                 bias_blk[:, 512 * n:512 * (n + 1)],
                                 start=True, stop=False)
            for k in range(KT):
                for n in range(2):
                    nc.tensor.matmul(pm[:, n, :], xT[:, k, 128 * m:128 * (m + 1)],
                                     wT[:, k, 512 * n:512 * (n + 1)],
                                     start=False, stop=(k == KT - 1))
            nc.scalar.activation(ot[:, m, :], pm[:].rearrange("p a b -> p (a b)"), Copy)
            if m % 2 == 1:  # store l-tiles (m-1, m): 1 MB, 4 KB rows
                nc.scalar.dma_start(
                    out_d[b, 256 * (m // 2):256 * (m // 2 + 1), :]
                    .rearrange("(m p) o -> p m o", p=128),
                    ot[:, m - 1:m + 1, :])

        # ---------------- pipeline ----------------
        # Build the per-sample transpose/elementwise work as unit lists;
        # sample b+1's units are interleaved into sample b's GEMM stream.
        def sample_units(b, wT, xT):
            units = []
            x_nat = x_tiles.pop(b)
            for c in range(NCH):
                if b == 0:
                    units.append(lambda j=c: (emit_exp(j), emit_mean_group(j, 0),
                                              emit_mean_group(j, 1)))
                units.append(lambda c_=c: (emit_scale(b, c_),
                                           emit_chunk_group(b, c_, 0, wT)))
                units.append(lambda c_=c: emit_chunk_group(b, c_, 1, wT))
                units.append(lambda m=c: emit_xT_group(b, m, x_nat, xT))
            return units

        def emit_loads(b):
            """issue sample b's input loads (SWDGE cast f32->bf16)."""
            if b > 0:
                x_tiles[b] = xnat_pool.tile([128, LT, I], BF16, tag="xnat",
                                            name=f"xn{b}")
                nc.gpsimd.dma_start(
                    x_tiles[b][:], x_d[b].rearrange("(m p) i -> p m i", p=128))
                for c in range(NCH):
                    nz_tiles[(b, c)] = nz_pool.tile([128, 2, I], BF16, tag="nz",
                                                    name=f"nz{b}{c}")
                    nc.gpsimd.dma_start(
                        nz_tiles[(b, c)][:],
                        nz_d[b, 256 * c:256 * (c + 1), :]
                        .rearrange("(q p) i -> p q i", p=128))

        wxT = {0: (wT_pool.tile([128, KT, O], BF16, tag="wT", name="wT0"),
                   xT_pool.tile([128, KT, L], BF16, tag="xT", name="xT0"))}

        # prologue: sample 0's transposes (+ mean/exp) run before any GEMM
        for u in sample_units(0, *wxT[0]):
            u()

        # bias block + PE warm burst right before the first GEMM: ~3.5us of
        # real (HAM-visible) matmuls so GEMMs start at 2.4 GHz.
        for n in range(2):
            pb = psum_mm.tile([128, 2, 512], F32, tag="pmm")
            nc.tensor.matmul(pb[:, 0, :], ones_b[:], bias_b[:, 512 * n:512 * (n + 1)],
                             start=True, stop=True)
            nc.vector.tensor_copy(bias_blk[:, 512 * n:512 * (n + 1)], pb[:, 0, :])
        pw = psum_mm.tile([128, 2, 512], F32, tag="pmm")
        for _ in range(8):
            nc.tensor.matmul(pw[:, 0, :], ident_b[:], bias_blk[:, 0:512],
                             start=True, stop=True)

        for b in range(SAMPLES):
            emit_loads(b + 1) if b + 1 < SAMPLES else None
            wT, xT = wxT.pop(b)
            if b + 1 < SAMPLES:
                wxT[b + 1] = (wT_pool.tile([128, KT, O], BF16, tag="wT",
                                           name=f"wT{b+1}"),
                              xT_pool.tile([128, KT, L], BF16, tag="xT",
                                           name=f"xT{b+1}"))
                units = sample_units(b + 1, *wxT[b + 1])
            else:
                units = []
            ot = out_pool.tile([128, LT, 512 * 2], F32, tag="out")
            # interleave: GEMM l-tile, then up to 3 transpose units of b+1
            ui = 0
            for m in range(LT):
                emit_gemm_tile(b, m, wT, xT, ot)
                take = 3 if m < LT - 1 else len(units) - ui
                for _ in range(take):
                    if ui < len(units):
                        units[ui]()
                        ui += 1

    _split_multi_waits(nc, mybir)
    return nc


def _get_nc(use_f32r=True):
    key = ("nc", use_f32r)
    if key not in _cache:
        _cache[key] = build_nc(use_f32r)
    return _cache[key]


def kernel(x, weight_mean, weight_logvar, bias, noise):
    from concourse import bass_utils

    x = np.ascontiguousarray(x, dtype=np.float32)
    noise = np.ascontiguousarray(noise, dtype=np.float32)
    weight_mean = np.ascontiguousarray(weight_mean, dtype=np.float32)
    weight_logvar = np.ascontiguousarray(weight_logvar, dtype=np.float32)
    bias = np.ascontiguousarray(bias, dtype=np.float32)

    nc = _get_nc()
    in_maps = []
    for c in range(N_CORES):
        sl = slice(SAMPLES * c, SAMPLES * (c + 1))
        in_maps.append({
            "x": x[sl], "noise": noise[sl],
            "weight_mean": weight_mean, "weight_logvar": weight_logvar,
            "bias": bias,
        })
    res = bass_utils.run_bass_kernel_spmd(nc, in_maps, list(range(N_CORES)))
    out = np.concatenate([res.results[c]["out"] for c in range(N_CORES)], axis=0)
    return out.astype(np.float32)


# revision 4
# speedup vs baseline: 1.2433x; 1.0338x over previous
"""BayesianLinear Trainium2 kernel, 8-core SPMD (data-parallel over batch).

Per-core computation (4 samples each):
    w_b = weight_mean + noise_b * exp(0.5 * weight_logvar)   (B,O,I)
    out_b = x_b @ w_b^T + bias                               (B,L,O)

Design (per core) — DMA-roofline oriented (~40 MB/core HBM traffic):
  - All matmul operands are bf16 (tolerance 2e-2; bf16 path lands ~4e-3):
    noise/x/mean are cast f32->bf16 *during* the DMA load (SWDGE cast),
    std = exp(0.5*logvar) is produced in bf16 by ACT directly.
  - bf16 PE transposes run at ~56 ns/block warm (FWL); grouped 8-deep per
    PSUM bank so each evac/add is one [128,8,128] op.
  - GEMM: per l-tile one 2-bank PSUM tile; each xT stationary is shared
    by two N=512 matmuls (both output halves), halving LDWEIGHTS.
    Bias is added during the PSUM->SBUF evac (DVE add vs a resident
    broadcast block), so the PE runs pure 216 ns N=512 matmuls.
  - Software pipelining: sample b+1's transpose groups are interleaved
    between sample b's GEMM l-tiles so the PE never has a >2us stretch
    without real matmuls (keeps the HAM clock at 2.4 GHz).  Sample 0's
    GEMM is emitted half-by-half so it can start once chunks 0,1 have
    landed (~8 MB in) instead of waiting for the full first sample.
"""
import numpy as np

SAMPLES = 4           # batch samples per core
N_CORES = 8
B, L, I, O = 32, 512, 1024, 1024
KT = I // 128         # 8 k-tiles (contraction)
OT = O // 128         # 8 o-blocks
LT = L // 128         # 4 l-tiles
NCH = 4               # noise chunks per sample (2 o-blocks each)

_cache = {}


def _split_multi_waits(nc, mybir):
    """This walrus build allows at most one sync-wait per instruction; move
    extra waits onto preceding single-wait NOPs on the same engine.  Safe
    because kernel semaphores are monotonic between resets, so waiting
    sequentially is equivalent to waiting on the conjunction."""
    for fn in nc.m.functions:
        for bb in fn.blocks:
            insts = bb.instructions
            changed = False
            new_list = []
            for inst in insts:
                si = inst.sync_info
                if si is not None and si.on_wait and len(si.on_wait) > 1:
                    waits = list(si.on_wait)
                    for j, w in enumerate(waits[:-1]):
                        nop = mybir.InstNoOp(name=f"{inst.name}-w{j}", ins=[], outs=[])
                        nop.engine = inst.engine
                        nop.sync_info = mybir.SyncInfo(on_wait=[w], on_update=[])
                        new_list.append(nop)
                    inst.sync_info = mybir.SyncInfo(
                        on_wait=[waits[-1]], on_update=list(si.on_update or []))
                    changed = True
                new_list.append(inst)
            if changed:
                bb.instructions = new_list


def build_nc(use_f32r=True):
    from contextlib import ExitStack
    from concourse import bass, mybir, tile, masks

    F32 = mybir.dt.float32
    BF16 = mybir.dt.bfloat16
    Exp = mybir.ActivationFunctionType.Exp
    Copy = mybir.ActivationFunctionType.Copy
    mult = mybir.AluOpType.mult
    add = mybir.AluOpType.add

    nc = bass.Bass()
    x_d = nc.declare_dram_parameter("x", [SAMPLES, L, I], F32, isOutput=False)
    nz_d = nc.declare_dram_parameter("noise", [SAMPLES, O, I], F32, isOutput=False)
    wm_d = nc.declare_dram_parameter("weight_mean", [O, I], F32, isOutput=False)
    wl_d = nc.declare_dram_parameter("weight_logvar", [O, I], F32, isOutput=False)
    b_d = nc.declare_dram_parameter("bias", [O], F32, isOutput=False)
    out_d = nc.declare_dram_parameter("out", [SAMPLES, L, O], F32, isOutput=True)

    with tile.TileContext(nc) as tc, ExitStack() as ctx:
        resident = ctx.enter_context(tc.tile_pool(name="resident", bufs=1))
        lv_pool = ctx.enter_context(tc.tile_pool(name="lv", bufs=2))
        mn_pool = ctx.enter_context(tc.tile_pool(name="mn", bufs=2))
        nz_pool = ctx.enter_context(tc.tile_pool(name="nz", bufs=5))
        sc_pool = ctx.enter_context(tc.tile_pool(name="scn", bufs=3))
        xnat_pool = ctx.enter_context(tc.tile_pool(name="xnat", bufs=2))
        xT_pool = ctx.enter_context(tc.tile_pool(name="xT", bufs=2))
        wT_pool = ctx.enter_context(tc.tile_pool(name="wT", bufs=2))
        out_pool = ctx.enter_context(tc.tile_pool(name="outp", bufs=2))
        psum_mm = ctx.enter_context(tc.tile_pool(name="psum_mm", bufs=2, space="PSUM"))
        psum_nt = ctx.enter_context(tc.tile_pool(name="psum_nt", bufs=2, space="PSUM"))
        psum_xt = ctx.enter_context(tc.tile_pool(name="psum_xt", bufs=2, space="PSUM"))

        # ---------------- residents ----------------
        std_b = resident.tile([128, OT, I], BF16, tag="std")     # exp(.5 lv), natural
        meanT = resident.tile([128, KT, O], BF16, tag="meanT")   # mean^T
        ident = resident.tile([128, 128], F32, tag="ident")
        ident_b = resident.tile([128, 128], BF16, tag="ident_b")
        ones_b = resident.tile([1, 128], BF16, tag="ones_b")
        bias_f = resident.tile([1, O], F32, tag="bias_f")
        bias_b = resident.tile([1, O], BF16, tag="bias_b")
        bias_blk = resident.tile([128, O], F32, tag="bias_blk")  # bias bcast to rows

        # ---------------- earliest DMA issues ----------------
        # SWDGE (gpsimd) order matters: first chunk's mean+noise, then x,
        # then the rest, matching the prologue's consumption order.
        nc.sync.dma_start(bias_f[:], b_d[:].rearrange("(a o) -> a o", a=1))
        mn_tiles, nz_tiles, lv_tiles = {}, {}, {}

        def emit_lv_load(j):
            lv_tiles[j] = lv_pool.tile([128, 2, I], F32, tag="lv", name=f"lv{j}")
            nc.sync.dma_start(
                lv_tiles[j][:],
                wl_d[256 * j:256 * (j + 1), :].rearrange("(q p) i -> p q i", p=128))

        def emit_mn_load(j):
            mn_tiles[j] = mn_pool.tile([128, 2, I], BF16, tag="mn", name=f"mn{j}")
            nc.gpsimd.dma_start(
                mn_tiles[j][:],
                wm_d[256 * j:256 * (j + 1), :].rearrange("(q p) i -> p q i", p=128))

        def emit_nz_load(b, c):
            nz_tiles[(b, c)] = nz_pool.tile([128, 2, I], BF16, tag="nz",
                                            name=f"nz{b}{c}")
            nc.gpsimd.dma_start(
                nz_tiles[(b, c)][:],
                nz_d[b, 256 * c:256 * (c + 1), :].rearrange("(q p) i -> p q i", p=128))

        def emit_x_load(b):
            x_tiles[b] = xnat_pool.tile([128, LT, I], BF16, tag="xnat", name=f"xn{b}")
            nc.gpsimd.dma_start(
                x_tiles[b][:], x_d[b].rearrange("(m p) i -> p m i", p=128))

        x_tiles = {}
        emit_lv_load(0), emit_lv_load(1)
        emit_mn_load(0), emit_nz_load(0, 0), emit_x_load(0)
        emit_mn_load(1), emit_nz_load(0, 1)
        emit_lv_load(2), emit_lv_load(3)
        emit_mn_load(2), emit_nz_load(0, 2)
        emit_mn_load(3), emit_nz_load(0, 3)

        # ---------------- setup compute ----------------
        masks.make_identity(nc, ident[:])
        nc.vector.tensor_copy(ident_b[:], ident[:])
        nc.vector.memset(ones_b[:], 1.0)
        nc.vector.tensor_copy(bias_b[:], bias_f[:])

        # ---------------- emitters ----------------
        def emit_mean_group(j, q):
            """transpose mean slab j, column-half q -> meanT o-block 2j+q."""
            ob = 2 * j + q
            mn = mn_tiles[j] if q == 0 else mn_tiles.pop(j)
            pt = psum_nt.tile([128, KT, 128], BF16, tag="pnt")
            for k in range(KT):
                nc.tensor.matmul(pt[:, k, :], mn[:, q, 128 * k:128 * (k + 1)],
                                 ident_b[:], is_transpose=True, start=True, stop=True)
            nc.scalar.activation(meanT[:, :, 128 * ob:128 * (ob + 1)], pt[:], Copy)

        def emit_exp(j):
            nc.scalar.activation(std_b[:, 2 * j:2 * (j + 1), :], lv_tiles.pop(j)[:],
                                 Exp, bias=0.0, scale=0.5)

        sc_tiles = {}

        def emit_scale(b, c):
            """sc = noise_chunk * std (bf16)."""
            nz = nz_tiles.pop((b, c))
            sc = sc_pool.tile([128, 2, I], BF16, tag="scn")
            nc.vector.tensor_tensor(sc[:], nz[:], std_b[:, 2 * c:2 * (c + 1), :], mult)
            sc_tiles[(b, c)] = sc

        def emit_chunk_group(b, c, q, wT):
            """transpose sc chunk c half q, add mean^T -> wT o-block 2c+q."""
            ob = 2 * c + q
            sc = sc_tiles[(b, c)] if q == 0 else sc_tiles.pop((b, c))
            pt = psum_nt.tile([128, KT, 128], BF16, tag="pnt")
            for k in range(KT):
                nc.tensor.matmul(pt[:, k, :], sc[:, q, 128 * k:128 * (k + 1)],
                                 ident_b[:], is_transpose=True, start=True, stop=True)
            nc.vector.tensor_tensor(wT[:, :, 128 * ob:128 * (ob + 1)], pt[:],
                                    meanT[:, :, 128 * ob:128 * (ob + 1)], add)

        def emit_xT_group(b, m, x_nat, xT):
            """transpose x l-tile m -> xT[:, :, 128m:128(m+1)]."""
            pt = psum_xt.tile([128, KT, 128], BF16, tag="pxt")
            for k in range(KT):
                nc.tensor.matmul(pt[:, k, :], x_nat[:, m, 128 * k:128 * (k + 1)],
                                 ident_b[:], is_transpose=True, start=True, stop=True)
            nc.scalar.activation(xT[:, :, 128 * m:128 * (m + 1)], pt[:], Copy)

        def emit_store(b, m, ot):
            if m % 2 == 1:  # store l-tiles (m-1, m): 1 MB, 4 KB rows
                nc.scalar.dma_start(
                    out_d[b, 256 * (m // 2):256 * (m // 2 + 1), :]
                    .rearrange("(m p) o -> p m o", p=128),
                    ot[:, m - 1:m + 1, :])

        def emit_gemm_tile(b, m, wT, xT, ot):
            """output l-tile m, all 1024 columns: 8 shared-stationary k-steps,
            two N=512 matmuls (psum banks) per step; bias added on evac."""
            pm = psum_mm.tile([128, 2, 512], F32, tag="pmm")
            for k in range(KT):
                for n in range(2):
                    nc.tensor.matmul(pm[:, n, :], xT[:, k, 128 * m:128 * (m + 1)],
                                     wT[:, k, 512 * n:512 * (n + 1)],
                                     start=(k == 0), stop=(k == KT - 1))
            nc.vector.tensor_tensor(ot[:, m, :], pm[:].rearrange("p a b -> p (a b)"),
                                    bias_blk[:], add)
            emit_store(b, m, ot)

        def emit_gemm_half(b, m, n, wT, xT, ot):
            """sample-0 prologue variant: one output half (512 cols)."""
            pm = psum_mm.tile([128, 2, 512], F32, tag="pmm")
            for k in range(KT):
                nc.tensor.matmul(pm[:, 0, :], xT[:, k, 128 * m:128 * (m + 1)],
                                 wT[:, k, 512 * n:512 * (n + 1)],
                                 start=(k == 0), stop=(k == KT - 1))
            nc.vector.tensor_tensor(ot[:, m, 512 * n:512 * (n + 1)], pm[:, 0, :],
                                    bias_blk[:, 512 * n:512 * (n + 1)], add)
            if n == 1:
                emit_store(b, m, ot)

        # ---------------- pipeline ----------------
        def chunk_units(b, c, wT, first_mean=False):
            u = []
            if first_mean:
                u.append(lambda: (emit_exp(c), emit_mean_group(c, 0),
                                  emit_mean_group(c, 1)))
            u.append(lambda: (emit_scale(b, c), emit_chunk_group(b, c, 0, wT)))
            u.append(lambda: emit_chunk_group(b, c, 1, wT))
            return u

        def emit_loads(b):
            emit_nz_load(b, 0)
            emit_nz_load(b, 1)
            emit_x_load(b)
            emit_nz_load(b, 2)
            emit_nz_load(b, 3)

        wxT = {0: (wT_pool.tile([128, KT, O], BF16, tag="wT", name="wT0"),
                   xT_pool.tile([128, KT, L], BF16, tag="xT", name="xT0"))}
        ots = {0: out_pool.tile([128, LT, 1024], F32, tag="out", name="ot0")}

        # ---- sample-0 prologue: chunks 0,1 + all x^T, then GEMM half 0 ----
        wT0, xT0 = wxT[0]
        x0 = x_tiles.pop(0)
        units = (chunk_units(0, 0, wT0, first_mean=True)
                 + [lambda: emit_xT_group(0, 0, x0, xT0),
                    lambda: emit_xT_group(0, 1, x0, xT0)]
                 + chunk_units(0, 1, wT0, first_mean=True)
                 + [lambda: emit_xT_group(0, 2, x0, xT0),
                    lambda: emit_xT_group(0, 3, x0, xT0)])
        for u in units:
            u()

        # bias block (via PE broadcast matmul) + warm burst right before the
        # first GEMM: ~3.5us of real (HAM-visible) matmuls -> 2.4 GHz clock.
        for n in range(2):
            pb = psum_mm.tile([128, 2, 512], F32, tag="pmm")
            nc.tensor.matmul(pb[:, 0, :], ones_b[:], bias_b[:, 512 * n:512 * (n + 1)],
                             start=True, stop=True)
            nc.scalar.activation(bias_blk[:, 512 * n:512 * (n + 1)], pb[:, 0, :], Copy)
        pw = psum_mm.tile([128, 2, 512], F32, tag="pmm")
        for _ in range(8):
            nc.tensor.matmul(pw[:, 0, :], ident_b[:], std_b[:, 0, 0:512],
                             start=True, stop=True)

        # GEMM sample 0 half 0, interleaved with chunks 2,3 (mean slabs 2,3)
        units = chunk_units(0, 2, wT0, first_mean=True) \
            + chunk_units(0, 3, wT0, first_mean=True)
        ui = 0
        for m in range(LT):
            emit_gemm_half(0, m, 0, wT0, xT0, ots[0])
            take = 2 if m < LT - 1 else len(units) - ui
            for _ in range(take):
                if ui < len(units):
                    units[ui]()
                    ui += 1

        # ---- main loop: sample b GEMM interleaved with sample b+1 prep ----
        for b in range(SAMPLES):
            if b + 1 < SAMPLES:
                emit_loads(b + 1)
                wxT[b + 1] = (wT_pool.tile([128, KT, O], BF16, tag="wT",
                                           name=f"wT{b+1}"),
                              xT_pool.tile([128, KT, L], BF16, tag="xT",
                                           name=f"xT{b+1}"))
                ots[b + 1] = out_pool.tile([128, LT, 1024], F32, tag="out",
                                           name=f"ot{b+1}")
                wTn, xTn = wxT[b + 1]
                xn = x_tiles.pop(b + 1)
                units = []
                for c in range(NCH):
                    units += chunk_units(b + 1, c, wTn)
                    units.append(lambda m=c: emit_xT_group(b + 1, m, xn, xTn))
            else:
                units = []
            wT, xT = wxT.pop(b)
            ot = ots.pop(b)
            ui = 0
            for m in range(LT):
                if b == 0:
                    emit_gemm_half(0, m, 1, wT, xT, ot)
                else:
                    emit_gemm_tile(b, m, wT, xT, ot)
                take = 3 if m < LT - 1 else len(units) - ui
                for _ in range(take):
                    if ui < len(units):
                        units[ui]()
                        ui += 1

    _split_multi_waits(nc, mybir)
    return nc


def _get_nc(use_f32r=True):
    key = ("nc", use_f32r)
    if key not in _cache:
        _cache[key] = build_nc(use_f32r)
    return _cache[key]


def kernel(x, weight_mean, weight_logvar, bias, noise):
    from concourse import bass_utils

    x = np.ascontiguousarray(x, dtype=np.float32)
    noise = np.ascontiguousarray(noise, dtype=np.float32)
    weight_mean = np.ascontiguousarray(weight_mean, dtype=np.float32)
    weight_logvar = np.ascontiguousarray(weight_logvar, dtype=np.float32)
    bias = np.ascontiguousarray(bias, dtype=np.float32)

    nc = _get_nc()
    in_maps = []
    for c in range(N_CORES):
        sl = slice(SAMPLES * c, SAMPLES * (c + 1))
        in_maps.append({
            "x": x[sl], "noise": noise[sl],
            "weight_mean": weight_mean, "weight_logvar": weight_logvar,
            "bias": bias,
        })
    res = bass_utils.run_bass_kernel_spmd(nc, in_maps, list(range(N_CORES)))
    out = np.concatenate([res.results[c]["out"] for c in range(N_CORES)], axis=0)
    return out.astype(np.float32)
